# revision 5
# baseline (speedup 1.0000x reference)
"""BlockCirculantLinear kernel for 8x TRN2 NeuronCores.

Math: the reference's per-block circular correlation via FFT is exactly a
dense matmul out = (x * D) @ M where M[j*b+s, o*b+t] = W[o, j, (s-t) mod b].
We expand M on the host (folding D into its rows), shard the batch dim of x
across 8 cores, and run a tiled matmul per core on the TensorEngine.

Matmul dtype is float32r: fp32 storage, PE truncates operands to the top 12
significand bits and streams at full rate (4x faster than fp32 mode).
Measured end-to-end relative error ~1.4e-4. Set SPLIT_EXACT=True for a
3-product hi/lo split that recovers fp32-level accuracy (~2e-7) at 3x the
matmul cost.

Per-core device program (SPMD, same NEFF on all 8 cores):
  inputs : xT  [4096, 1024] f32  (x_shard transposed on host; K on rows)
           mt_in [32, 128, 32, 128] f32 (expanded M, pre-tiled: [nt, p, ko, t])
  output : outT [4096, 1024] f32 (out_shard transposed; host transposes back)

  x is cached fully in SBUF (16 MB, 4 tiles for fine-grained deps). M streams
  through SBUF in 128-column chunks (contiguous 2 MB DMAs); for each chunk,
  psum[n(128), m(0:512 / 512:1024)] accumulates over the 32 k-tiles with
  lhsT = M-tile (stationary), rhs = x-tile (moving).
"""

import numpy as np

B_TOTAL = 8192
D_IN = 4096
D_OUT = 4096
N_CORES = 8
B_SHARD = B_TOTAL // N_CORES  # 1024

P = 128
KO = D_IN // P               # 32 k-tiles of 128
XC_SPLIT = 4                 # x-cache tiles (KO/XC_SPLIT k-tiles each)
KO_PER_XC = KO // XC_SPLIT
N_TILE = 128                 # columns of M per inner chunk
N_TILES = D_OUT // N_TILE    # 32
MM_FREE = 512                # moving free dim per matmul (one PSUM bank)
M_CHUNKS = B_SHARD // MM_FREE  # 2

MM_DTYPE = "float32r"
SPLIT_EXACT = False

_compiled = None


def _expand_M(W: np.ndarray, D: np.ndarray) -> np.ndarray:
    """Pre-tiled dense M: out[nt, p, ko, t] = M[ko*128+p, nt*128+t] where
    M[j*b+s, o*b+t] = D[j*b+s] * W[o, j, (s-t) mod b]."""
    k_out, k_in, b = W.shape
    s = np.arange(b)[:, None]
    t = np.arange(b)[None, :]
    idx = (s - t) % b
    circ = W[:, :, idx]                          # [k_out(o), k_in(j), b(s), b(t)]
    Dm = D.reshape(k_in, b)                      # [j, s]
    circ = circ * Dm[None, :, :, None]
    # want [nt=o, p=s, ko=j, t]
    mt = circ.transpose(0, 2, 1, 3)              # [o, s, j, t]
    return np.ascontiguousarray(mt, dtype=np.float32)


def _build_module():
    import concourse.bass as bass
    import concourse.tile as tile
    from concourse import bacc, mybir

    nc = bacc.Bacc("TRN2", target_bir_lowering=False, debug=False)

    mm_dt = getattr(mybir.dt, MM_DTYPE)
    f32 = mybir.dt.float32

    xT = nc.dram_tensor("xT", [D_IN, B_SHARD], mm_dt, kind="ExternalInput")
    mt_in = nc.dram_tensor(
        "mt_in", [N_TILES, P, KO, N_TILE], mm_dt, kind="ExternalInput"
    )
    outT = nc.dram_tensor("outT", [D_OUT, B_SHARD], f32, kind="ExternalOutput")

    xT_v = xT.rearrange("(ko p) m -> p ko m", p=P)    # [128, 32, 1024]

    with tile.TileContext(nc) as tc:
        with (
            tc.tile_pool(name="xcache", bufs=1) as xpool,
            tc.tile_pool(name="mtiles", bufs=3) as mpool,
            tc.tile_pool(name="otiles", bufs=3) as opool,
            tc.tile_pool(name="psum", bufs=4, space="PSUM") as psum_pool,
        ):
            xcs = []
            for xi in range(XC_SPLIT):
                xc = xpool.tile([P, KO_PER_XC, B_SHARD], mm_dt, name=f"xc{xi}")
                nc.sync.dma_start(
                    xc[:], xT_v[:, xi * KO_PER_XC : (xi + 1) * KO_PER_XC, :]
                )
                xcs.append(xc)

            for nt in range(N_TILES):
                mt = mpool.tile([P, KO, N_TILE], mm_dt, tag="mt", name=f"mt{nt}")
                nc.sync.dma_start(mt[:], mt_in[nt])
                psums = [
                    psum_pool.tile([P, MM_FREE], f32, tag=f"ps{i}", name=f"ps{i}_{nt}")
                    for i in range(M_CHUNKS)
                ]
                for ko in range(KO):
                    xc = xcs[ko // KO_PER_XC]
                    kk = ko % KO_PER_XC
                    for mc in range(M_CHUNKS):
                        nc.tensor.matmul(
                            psums[mc][:],
                            lhsT=mt[:, ko, :],
                            rhs=xc[:, kk, mc * MM_FREE : (mc + 1) * MM_FREE],
                            start=(ko == 0),
                            stop=(ko == KO - 1),
                        )
                ot = opool.tile([P, B_SHARD], f32, tag="ot", name=f"ot{nt}")
                for mc in range(M_CHUNKS):
                    nc.vector.tensor_copy(
                        ot[:, mc * MM_FREE : (mc + 1) * MM_FREE], psums[mc][:]
                    )
                nc.sync.dma_start(outT[nt * N_TILE : (nt + 1) * N_TILE, :], ot[:])

    nc.compile()
    return nc


def _build_module_split():
    """Exact variant: hi/lo split of both operands, 3 f32r products per
    accumulation group. Error ~2e-7 (fp32 envelope)."""
    import concourse.bass as bass
    import concourse.tile as tile
    from concourse import bacc, mybir

    nc = bacc.Bacc("TRN2", target_bir_lowering=False, debug=False)

    f32r = mybir.dt.float32r
    f32 = mybir.dt.float32

    # host supplies hi/lo pre-split tensors
    xT_h = nc.dram_tensor("xT_h", [D_IN, B_SHARD], f32r, kind="ExternalInput")
    xT_l = nc.dram_tensor("xT_l", [D_IN, B_SHARD], f32r, kind="ExternalInput")
    mt_h = nc.dram_tensor(
        "mt_h", [N_TILES, P, KO, N_TILE], f32r, kind="ExternalInput"
    )
    mt_l = nc.dram_tensor(
        "mt_l", [N_TILES, P, KO, N_TILE], f32r, kind="ExternalInput"
    )
    outT = nc.dram_tensor("outT", [D_OUT, B_SHARD], f32, kind="ExternalOutput")

    xh_v = xT_h.rearrange("(ko p) m -> p ko m", p=P)
    xl_v = xT_l.rearrange("(ko p) m -> p ko m", p=P)

    with tile.TileContext(nc) as tc:
        with (
            tc.tile_pool(name="xcache", bufs=1) as xpool,
            tc.tile_pool(name="mtiles", bufs=3) as mpool,
            tc.tile_pool(name="otiles", bufs=3) as opool,
            tc.tile_pool(name="psum", bufs=4, space="PSUM") as psum_pool,
        ):
            xhs, xls = [], []
            for xi in range(XC_SPLIT):
                ksl = slice(xi * KO_PER_XC, (xi + 1) * KO_PER_XC)
                xh = xpool.tile([P, KO_PER_XC, B_SHARD], f32r, name=f"xh{xi}")
                nc.sync.dma_start(xh[:], xh_v[:, ksl, :])
                xhs.append(xh)
                xl = xpool.tile([P, KO_PER_XC, B_SHARD], f32r, name=f"xl{xi}")
                nc.sync.dma_start(xl[:], xl_v[:, ksl, :])
                xls.append(xl)

            for nt in range(N_TILES):
                mh = mpool.tile([P, KO, N_TILE], f32r, tag="mh", name=f"mh{nt}")
                nc.sync.dma_start(mh[:], mt_h[nt])
                ml = mpool.tile([P, KO, N_TILE], f32r, tag="ml", name=f"ml{nt}")
                nc.sync.dma_start(ml[:], mt_l[nt])
                psums = [
                    psum_pool.tile([P, MM_FREE], f32, tag=f"ps{i}", name=f"ps{i}_{nt}")
                    for i in range(M_CHUNKS)
                ]
                n_steps = KO * 3
                step = 0
                for ko in range(KO):
                    xh = xhs[ko // KO_PER_XC]
                    xl = xls[ko // KO_PER_XC]
                    kk = ko % KO_PER_XC
                    for wt, xt in ((mh, xh), (ml, xh), (mh, xl)):
                        for mc in range(M_CHUNKS):
                            nc.tensor.matmul(
                                psums[mc][:],
                                lhsT=wt[:, ko, :],
                                rhs=xt[:, kk, mc * MM_FREE : (mc + 1) * MM_FREE],
                                start=(step == 0),
                                stop=(step == n_steps - 1),
                            )
                        step += 1
                ot = opool.tile([P, B_SHARD], f32, tag="ot", name=f"ot{nt}")
                for mc in range(M_CHUNKS):
                    nc.vector.tensor_copy(
                        ot[:, mc * MM_FREE : (mc + 1) * MM_FREE], psums[mc][:]
                    )
                nc.sync.dma_start(outT[nt * N_TILE : (nt + 1) * N_TILE, :], ot[:])

    nc.compile()
    return nc


def _get_module():
    global _compiled
    if _compiled is None:
        _compiled = _build_module_split() if SPLIT_EXACT else _build_module()
    return _compiled


def _trunc_hi(a: np.ndarray) -> np.ndarray:
    """Top 11 explicit mantissa bits (the part f32r multiplies exactly)."""
    u = a.view(np.uint32)
    return (u & np.uint32(0xFFFFF000)).view(np.float32)


def kernel(x: np.ndarray, W: np.ndarray, D_bernoulli: np.ndarray) -> np.ndarray:
    from concourse.bass_utils import run_bass_kernel_spmd

    x = np.ascontiguousarray(np.asarray(x, dtype=np.float32))
    mt = _expand_M(np.asarray(W, np.float32), np.asarray(D_bernoulli, np.float32))

    in_maps = []
    if SPLIT_EXACT:
        mh = _trunc_hi(mt)
        ml = mt - mh
        for c in range(N_CORES):
            xs = np.ascontiguousarray(x[c * B_SHARD : (c + 1) * B_SHARD].T)
            xh = _trunc_hi(xs)
            xl = xs - xh
            in_maps.append({"xT_h": xh, "xT_l": xl, "mt_h": mh, "mt_l": ml})
    else:
        for c in range(N_CORES):
            xs = x[c * B_SHARD : (c + 1) * B_SHARD]
            in_maps.append({"xT": np.ascontiguousarray(xs.T), "mt_in": mt})

    nc = _get_module()
    res = run_bass_kernel_spmd(nc, in_maps, core_ids=list(range(N_CORES)))
    out = np.empty((B_TOTAL, D_OUT), dtype=np.float32)
    for c in range(N_CORES):
        out[c * B_SHARD : (c + 1) * B_SHARD] = res.results[c]["outT"].T
    return out
